# revision 2
# baseline (speedup 1.0000x reference)
"""PointGroup clusters_voxelization v2: PE one-hot feat extraction + Pool
indirect gather running concurrently; coords via a cluster-contiguous
host-prepped stream (one plain DMA + per-partition stats/params/transform).

Static shared program (SPMD across 8 cores):
  - PE side: chunks [0, TC) of 128 table rows each; 32 slots per chunk;
    per 12-chunk group: build one-hot moving block [128, 384] bf16 via
    is_equal(rw_bcast, iota), 12 matmuls (stationary = interleaved feat
    chunk [128, 32] bf16) into one PSUM bank [32, 384] f32, cast-copy to
    bf16, write planes [32, TC*32].  Sorted-by-pid slot assignment with
    per-chunk overflow spilled to the Pool side (host-prepped).
  - Pool side: POOL_CAP points (chunk range [TC, NCH) + PE overflow + pad)
    gathered as bf16 feat rows via indirect DMA, 128/instruction.
  - Coord side: host ships coords[pid] bf16 (original order is cluster-
    contiguous): partition c = cluster c; min/max reduce, baseline params
    algebra, per-partition transform, f32 write.
Host reassembles: feats by slot/pool position maps, coords by reshape.
"""
import numpy as np
import ml_dtypes

import concourse.bass as bass
import concourse.bacc as bacc
import concourse.tile as tile
import concourse.mybir as mybir
from concourse import bass_utils

N = 1048576
C = 32
NCLUSTER = 1024
PTS = 2048
S = NCLUSTER * PTS
NCORES = 8
PPC = S // NCORES            # 262144 points per core
P = 128
NCH = N // P                 # 8192 chunks of 128 table rows
SL = 32                      # slots per chunk (PE side)
GC = 12                      # chunks per psum group
GCOL = GC * SL               # 384 cols per group
TC = 5244                    # chunks handled by PE (rest -> pool)
NGRP = TC // GC              # 437
PECOLS = TC * SL             # 167808
POOL_CAP = 114688            # pool-side points (896 instructions)
POOL_I = POOL_CAP // P

f32 = mybir.dt.float32
bf16 = mybir.dt.bfloat16
i32 = mybir.dt.int32

_CACHE = {}


def _build_program(fs, sc):
    key = (fs, sc)
    if key in _CACHE:
        return _CACHE[key]
    nc = bacc.Bacc("TRN2", target_bir_lowering=False, debug=False)
    tbl_d = nc.dram_tensor("tbl", (P, NCH, C), bf16, kind="ExternalInput")
    frow_d = nc.dram_tensor("frow", (N, C), bf16, kind="ExternalInput")
    rw_d = nc.dram_tensor("rw", (PECOLS,), bf16, kind="ExternalInput")
    iota_d = nc.dram_tensor("iota", (P, GCOL), bf16, kind="ExternalInput")
    ppid_d = nc.dram_tensor("ppid", (POOL_CAP,), i32, kind="ExternalInput")
    cst_d = nc.dram_tensor("cst", (P, PTS * 3), bf16, kind="ExternalInput")
    jit_d = nc.dram_tensor("jit", (2, 3), f32, kind="ExternalInput")
    outf_d = nc.dram_tensor("outf", (C, PECOLS), bf16, kind="ExternalOutput")
    outp_d = nc.dram_tensor("outp", (POOL_CAP, C), bf16,
                            kind="ExternalOutput")
    outc_d = nc.dram_tensor("outc", (P, PTS * 3), f32, kind="ExternalOutput")

    with tile.TileContext(nc) as tc:
        with (
            tc.tile_pool(name="one", bufs=1) as one,
            tc.tile_pool(name="tg", bufs=3) as tgp,
            tc.tile_pool(name="mv", bufs=3) as mvp,
            tc.tile_pool(name="ps", bufs=4, space="PSUM") as psp,
            tc.tile_pool(name="sb", bufs=3) as sbp,
            tc.tile_pool(name="sm", bufs=2) as smp,
            tc.tile_pool(name="pg", bufs=4) as pgp,
        ):
            iota_t = one.tile([P, GCOL], bf16)
            nc.sync.dma_start(out=iota_t[:], in_=iota_d.ap())
            pidx_t = one.tile([P, POOL_I], i32)
            nc.sync.dma_start(
                out=pidx_t[:],
                in_=bass.AP(tensor=ppid_d, offset=0,
                            ap=[[1, P], [P, POOL_I]]),
            )

            # ---------- coord pipeline ----------
            ct = one.tile([P, PTS * 3], bf16)
            nc.sync.dma_start(out=ct[:], in_=cst_d.ap())
            jit_t = one.tile([P, 6], f32)
            jsrc = jit_d.ap().rearrange("a b -> (a b)")
            nc.sync.dma_start(
                out=jit_t[:],
                in_=bass.AP(tensor=jsrc.tensor, offset=jsrc.offset,
                            ap=[[0, P]] + jsrc.ap),
            )
            st = smp.tile([P, 8], f32, name="st")
            ct_ap = ct[:]
            comp_ap = bass.AP(tensor=ct_ap.tensor, offset=ct_ap.offset,
                              ap=[ct_ap.ap[0], [1, 3], [3, PTS]])
            nc.vector.tensor_reduce(
                out=st[:, 0:3], in_=comp_ap,
                axis=mybir.AxisListType.X, op=mybir.AluOpType.min)
            nc.vector.tensor_reduce(
                out=st[:, 3:6], in_=comp_ap,
                axis=mybir.AxisListType.X, op=mybir.AluOpType.max)
            pr = smp.tile([P, 16], f32, name="pr")
            WD, T0, T1, MS = (slice(0, 3), slice(3, 6), slice(6, 9),
                              slice(9, 12))
            sc_t = smp.tile([P, 4], f32, name="sc_t")
            nc.vector.tensor_tensor(out=pr[:, WD], in0=st[:, 3:6],
                                    in1=st[:, 0:3],
                                    op=mybir.AluOpType.subtract)
            nc.vector.reduce_max(out=sc_t[:, 0:1], in_=pr[:, WD],
                                 axis=mybir.AxisListType.X)
            nc.vector.reciprocal(out=sc_t[:, 1:2], in_=sc_t[:, 0:1])
            nc.vector.tensor_scalar(
                out=sc_t[:, 2:3], in0=sc_t[:, 1:2], scalar1=fs, scalar2=-0.01,
                op0=mybir.AluOpType.mult, op1=mybir.AluOpType.add)
            nc.vector.tensor_scalar(
                out=sc_t[:, 2:3], in0=sc_t[:, 2:3], scalar1=sc, scalar2=None,
                op0=mybir.AluOpType.min)
            s_ap = sc_t[:, 2:3]
            nc.vector.tensor_scalar(
                out=pr[:, T0], in0=pr[:, WD], scalar1=s_ap, scalar2=None,
                op0=mybir.AluOpType.mult)
            nc.vector.tensor_scalar(
                out=pr[:, T0], in0=pr[:, T0], scalar1=-1.0, scalar2=fs,
                op0=mybir.AluOpType.mult, op1=mybir.AluOpType.add)
            nc.vector.tensor_scalar(
                out=pr[:, T1], in0=pr[:, T0], scalar1=0.001, scalar2=0.0,
                op0=mybir.AluOpType.add, op1=mybir.AluOpType.min)
            nc.vector.tensor_scalar(
                out=pr[:, T0], in0=pr[:, T0], scalar1=-0.001, scalar2=0.0,
                op0=mybir.AluOpType.add, op1=mybir.AluOpType.max)
            nc.vector.tensor_tensor(out=pr[:, T0], in0=pr[:, T0],
                                    in1=jit_t[:, 0:3],
                                    op=mybir.AluOpType.mult)
            nc.vector.tensor_tensor(out=pr[:, T1], in0=pr[:, T1],
                                    in1=jit_t[:, 3:6],
                                    op=mybir.AluOpType.mult)
            nc.vector.tensor_scalar(
                out=pr[:, MS], in0=st[:, 0:3], scalar1=s_ap, scalar2=None,
                op0=mybir.AluOpType.mult)
            prm = smp.tile([P, 4], f32, name="prm")
            nc.vector.tensor_copy(out=prm[:, 0:1], in_=s_ap)
            nc.vector.tensor_tensor(out=pr[:, T0], in0=pr[:, T0],
                                    in1=pr[:, T1], op=mybir.AluOpType.add)
            nc.vector.tensor_tensor(out=prm[:, 1:4], in0=pr[:, T0],
                                    in1=pr[:, MS],
                                    op=mybir.AluOpType.subtract)
            cot = one.tile([P, PTS * 3], f32)
            cot_ap = cot[:]
            for comp in range(3):
                src = bass.AP(tensor=ct_ap.tensor, offset=ct_ap.offset + comp,
                              ap=[ct_ap.ap[0], [3, PTS]])
                dst = bass.AP(tensor=cot_ap.tensor,
                              offset=cot_ap.offset + comp,
                              ap=[cot_ap.ap[0], [3, PTS]])
                nc.vector.tensor_scalar(
                    out=dst, in0=src,
                    scalar1=prm[:, 0:1], scalar2=prm[:, 1 + comp:2 + comp],
                    op0=mybir.AluOpType.mult, op1=mybir.AluOpType.add)
            nc.sync.dma_start(out=outc_d.ap(), in_=cot[:])

            # ---------- pool + PE, interleaved so both engines stream ----
            PB = 8  # pool instructions batched per write
            pool_batches = POOL_I // PB  # 112
            # interleave: per PE group, ~0.26 pool batches; emit pool batch
            # every ~4 PE groups
            pool_emitted = 0

            def emit_pool_batch(b):
                asm = pgp.tile([P, PB, C], bf16, name="pasm")
                for j in range(PB):
                    i = b * PB + j
                    nc.gpsimd.indirect_dma_start(
                        out=asm[:, j, :],
                        out_offset=None,
                        in_=frow_d.ap(),
                        in_offset=bass.IndirectOffsetOnAxis(
                            ap=pidx_t[:, i:i + 1], axis=0),
                    )
                nc.sync.dma_start(
                    out=bass.AP(tensor=outp_d, offset=b * PB * P * C,
                                ap=[[C, P], [P * C, PB], [1, C]]),
                    in_=asm[:],
                )

            for g in range(NGRP):
                tg = tgp.tile([P, GC, C], bf16, name="tg")
                nc.sync.dma_start(
                    out=tg[:],
                    in_=bass.AP(tensor=tbl_d, offset=g * GC * C,
                                ap=[[NCH * C, P], [C, GC], [1, C]]),
                )
                rwb = mvp.tile([P, GCOL], bf16, name="rwb")
                nc.scalar.dma_start(
                    out=rwb[:],
                    in_=bass.AP(tensor=rw_d, offset=g * GCOL,
                                ap=[[0, P], [1, GCOL]]),
                )
                mov = mvp.tile([P, GCOL], bf16, name="mov")
                nc.vector.tensor_tensor(
                    out=mov[:], in0=rwb[:], in1=iota_t[:],
                    op=mybir.AluOpType.is_equal)
                acc = psp.tile([C, GCOL], f32, name="acc")
                for i in range(GC):
                    nc.tensor.matmul(
                        acc[:, i * SL:(i + 1) * SL], tg[:, i, :],
                        mov[:, i * SL:(i + 1) * SL],
                        start=True, stop=True)
                sb = sbp.tile([C, GCOL], bf16, name="sb")
                nc.scalar.copy(out=sb[:], in_=acc[:])
                nc.sync.dma_start(
                    out=bass.AP(tensor=outf_d, offset=g * GCOL,
                                ap=[[PECOLS, C], [1, GCOL]]),
                    in_=sb[:],
                )
                want = ((g + 1) * pool_batches) // NGRP
                while pool_emitted < want:
                    emit_pool_batch(pool_emitted)
                    pool_emitted += 1
            while pool_emitted < pool_batches:
                emit_pool_batch(pool_emitted)
                pool_emitted += 1

    nc.compile()
    _CACHE[key] = nc
    return nc


def _reference_numpy(clusters_idx, clusters_offset, feats, coords, jitter,
                     fullscale, scale):
    seg = clusters_idx[:, 0].astype(np.int64)
    pid = clusters_idx[:, 1].astype(np.int64)
    nC = clusters_offset.shape[0] - 1
    fs = np.float32(fullscale)
    cf = feats[pid]
    cc = coords[pid].astype(np.float32)
    cnt = np.diff(clusters_offset).astype(np.float32)[:, None]
    sums = np.zeros((nC, 3), np.float32)
    np.add.at(sums, seg, cc)
    cmean = sums / np.maximum(cnt, 1.0)
    ccc = cc - cmean[seg]
    cmin = np.full((nC, 3), np.inf, np.float32)
    cmax = np.full((nC, 3), -np.inf, np.float32)
    np.minimum.at(cmin, seg, ccc)
    np.maximum.at(cmax, seg, ccc)
    cscale = 1.0 / ((cmax - cmin) / fs).max(axis=1) - np.float32(0.01)
    cscale = np.minimum(cscale, np.float32(scale)).astype(np.float32)
    mn = cmin * cscale[:, None]
    mx = cmax * cscale[:, None]
    ccc = ccc * cscale[seg][:, None]
    rng = mx - mn
    off = (-mn + np.maximum(fs - rng - 0.001, 0.0) * jitter[0]
           + np.minimum(fs - rng + 0.001, 0.0) * jitter[1]).astype(np.float32)
    ccc = ccc + off[seg]
    return np.concatenate([cf, ccc], axis=1).astype(np.float32)


def _prep_core(pid):
    """Slot assignment for one core.  Returns (rw, ppid, pe_pos, pool_pos)."""
    order = np.argsort(pid, kind="stable")
    srt = pid[order]
    ch = (srt >> 7).astype(np.int64)
    rwv = (srt & 127).astype(np.int64)
    # rank within chunk
    cum = np.concatenate([[0], np.cumsum(np.bincount(ch, minlength=NCH))])
    rank = np.arange(pid.size) - cum[ch]
    pe_mask = (ch < TC) & (rank < SL)
    slot = ch * SL + rank                      # valid where pe_mask
    rw = np.zeros(PECOLS, dtype=np.float32)
    pe_pos = np.full(PECOLS, -1, dtype=np.int64)
    rw[slot[pe_mask]] = rwv[pe_mask]
    pe_pos[slot[pe_mask]] = order[pe_mask]
    pool_sel = ~pe_mask
    ppid_v = srt[pool_sel]
    pool_pos_v = order[pool_sel]
    npool = ppid_v.size
    if npool > POOL_CAP:
        return None
    ppid = np.zeros(POOL_CAP, dtype=np.int32)
    pool_pos = np.full(POOL_CAP, -1, dtype=np.int64)
    ppid[:npool] = ppid_v.astype(np.int32)
    pool_pos[:npool] = pool_pos_v
    return rw.astype(ml_dtypes.bfloat16), ppid, pe_pos, pool_pos


def _make_in_maps(clusters_idx, feats, coords, jitter):
    """Build per-core input maps + position maps (shared with kernel())."""
    pid_full = np.asarray(clusters_idx)[:, 1].astype(np.int64)
    feats_bf = np.asarray(feats, np.float32).astype(ml_dtypes.bfloat16)
    tbl = np.ascontiguousarray(
        feats_bf.reshape(NCH, P, C).transpose(1, 0, 2))
    frow = np.ascontiguousarray(feats_bf)
    iota = np.broadcast_to(
        np.arange(P, dtype=np.float32).astype(
            ml_dtypes.bfloat16)[:, None], (P, GCOL)).copy()
    coords_bf = np.asarray(coords, np.float32).astype(ml_dtypes.bfloat16)
    in_maps, preps = [], []
    for k in range(NCORES):
        pid = pid_full[k * PPC:(k + 1) * PPC]
        pr = _prep_core(pid)
        if pr is None:
            return None, None
        rw, ppid, pe_pos, pool_pos = pr
        cst = np.ascontiguousarray(coords_bf[pid].reshape(P, PTS * 3))
        in_maps.append({
            "tbl": tbl, "frow": frow, "rw": rw, "iota": iota,
            "ppid": ppid, "cst": cst, "jit": np.asarray(jitter, np.float32),
        })
        preps.append((pe_pos, pool_pos))
    return in_maps, preps


def kernel(clusters_idx, clusters_offset, feats, coords, jitter, fullscale,
           scale):
    clusters_idx = np.asarray(clusters_idx)
    clusters_offset = np.asarray(clusters_offset)
    feats = np.asarray(feats, dtype=np.float32)
    coords = np.asarray(coords, dtype=np.float32)
    jitter = np.asarray(jitter, dtype=np.float32)
    fs = float(np.asarray(fullscale).item()) if not isinstance(
        fullscale, (int, float)) else float(fullscale)
    sc = float(np.asarray(scale).item()) if not isinstance(
        scale, (int, float)) else float(scale)

    uniform = (
        clusters_idx.shape == (S, 2)
        and clusters_offset.shape == (NCLUSTER + 1,)
        and feats.shape == (N, C)
        and coords.shape == (N, 3)
        and np.array_equal(clusters_offset,
                           np.arange(NCLUSTER + 1, dtype=np.int64) * PTS)
        and np.array_equal(clusters_idx[:, 0],
                           np.repeat(np.arange(NCLUSTER, dtype=np.int64),
                                     PTS))
    )
    if not uniform:
        return _reference_numpy(clusters_idx, clusters_offset, feats, coords,
                                jitter, fs, sc)

    in_maps, preps = _make_in_maps(clusters_idx, feats, coords, jitter)
    if in_maps is None:
        return _reference_numpy(clusters_idx, clusters_offset, feats,
                                coords, jitter, fs, sc)

    nc = _build_program(fs, sc)
    res = bass_utils.run_bass_kernel_spmd(nc, in_maps,
                                          core_ids=list(range(NCORES)))

    parts = []
    for k in range(NCORES):
        of = res.results[k]["outf"]          # (C, PECOLS) bf16
        op = res.results[k]["outp"]          # (POOL_CAP, C) bf16
        oc = res.results[k]["outc"]          # (P, PTS*3) f32
        pe_pos, pool_pos = preps[k]
        featp = np.empty((PPC, C), np.float32)
        m = pe_pos >= 0
        featp[pe_pos[m]] = of.T[m].astype(np.float32)
        m2 = pool_pos >= 0
        featp[pool_pos[m2]] = op[m2].astype(np.float32)
        coordp = oc.reshape(PPC, 3)
        parts.append(np.concatenate([featp, coordp], axis=1))
    return np.concatenate(parts, axis=0).astype(np.float32)
